# revision 12
# baseline (speedup 1.0000x reference)
"""MoE (top-2 of 8 experts) Trainium2 kernel.

Strategy: expert-parallel across the 8 NeuronCores. The router
(softmax + top-2 over [T, 8] logits) is metadata computed on host to
build the dispatch; core e receives only the tokens routed to expert e
(gathered, transposed, zero-padded to a common capacity C) plus that
expert's weights, pre-transposed so the device does no transposes:

  core e inputs:  xT  [H, C]   = x[idx_e].T        (padded)
                  w1T [H, I]   = w1[e].T
                  w2T [I, H]   = w2[e].T
                  gates [128, C/128]  renormalized top-2 weight per token
  core e output:  y   [C, H]   = gate * (silu(x_e @ w1[e].T) @ w2[e].T)

On device (per core, all fp32 storage, float32r matmuls):
  stage 1: hT[i_tile, c_chunk] = silu(w1T.T @ xT)   (I on partitions)
  stage 2: y[c_tile, h_chunk]  = hT.T @ w2T, scaled per-partition by gate

The host then scatter-adds the two expert contributions per token.
"""

import numpy as np

import concourse.bass as bass
import concourse.mybir as mybir
from concourse import bacc
from concourse.tile import TileContext
from concourse.bass_utils import run_bass_kernel_spmd

T, H, I, E = 4096, 1024, 1408, 8
TOPK = 2
P = 128
CHUNK = 512
N_CORES = 8
F32 = mybir.dt.float32
F32R = mybir.dt.float32r
AF = mybir.ActivationFunctionType

# most recently built device program (for test harnesses / cost-model timing)
LAST_NC = None


def _chunk_sizes(C):
    sizes = [CHUNK] * (C // CHUNK)
    if C % CHUNK:
        sizes.append(C % CHUNK)
    return sizes


def build_moe_expert_kernel(C, h=H, i_dim=I):
    """One-expert MLP over C gathered tokens. h, i_dim overridable for
    small-scale simulation tests; both must be multiples of 128, C a
    multiple of 128."""
    assert C % P == 0 and h % P == 0 and i_dim % P == 0
    HK = h // P
    IT = i_dim // P
    n_ct = C // P

    nc = bacc.Bacc("TRN2", target_bir_lowering=False, debug=False, num_devices=N_CORES)
    # Matmul inputs are stored as float32r (same 32-bit layout; the PE
    # rounds to its reduced internal precision). Typing the whole producer
    # chain as f32r satisfies the BIR verifier's rounding check.
    xT = nc.dram_tensor("xT", [h, C], F32R, kind="ExternalInput").ap()
    w1T = nc.dram_tensor("w1T", [h, i_dim], F32R, kind="ExternalInput").ap()
    w2T = nc.dram_tensor("w2T", [i_dim, h], F32R, kind="ExternalInput").ap()
    gates = nc.dram_tensor("gates", [P, n_ct], F32, kind="ExternalInput").ap()
    y = nc.dram_tensor("y", [C, h], F32, kind="ExternalOutput").ap()

    xT_v = xT.rearrange("(ho p) c -> p ho c", p=P)  # [128, HK, C]
    w1T_v = w1T.rearrange("(ho p) i -> p ho i", p=P)  # [128, HK, I]
    w2T_v = w2T.rearrange("(io p) h -> p io h", p=P)  # [128, IT, H]
    y_v = y.rearrange("(ct p) h -> ct p h", p=P)  # [n_ct, 128, H]

    h_chunks = _chunk_sizes(h)  # h-chunks for stage 2 output
    with TileContext(nc) as tc:
        with (
            tc.tile_pool(name="wpool", bufs=1) as wpool,
            tc.tile_pool(name="xpool", bufs=2) as xpool,
            tc.tile_pool(name="hpool", bufs=2) as hpool,
            tc.tile_pool(name="ypool", bufs=4) as ypool,
            tc.tile_pool(name="sgpool", bufs=2) as sgpool,
            tc.tile_pool(name="ps1", bufs=3, space="PSUM") as ps1pool,
            tc.tile_pool(name="ps2", bufs=3, space="PSUM") as ps2pool,
        ):
            gsb = wpool.tile([P, n_ct], F32)
            nc.sync.dma_start(gsb[:], gates[:])
            w1s = wpool.tile([P, HK, i_dim], F32R)
            # split the weight loads per i-tile so stage-1 compute can
            # start as soon as its first slice lands
            for it in range(IT):
                nc.sync.dma_start(
                    w1s[:, :, it * P : (it + 1) * P],
                    w1T_v[:, :, it * P : (it + 1) * P],
                )
            w2s = wpool.tile([P, IT, h], F32R)
            for it in range(IT):
                nc.sync.dma_start(w2s[:, it], w2T_v[:, it])

            c0 = 0
            for cs in _chunk_sizes(C):
                ct0 = c0 // P
                xs = xpool.tile([P, HK, CHUNK], F32R, tag="xs")
                nc.sync.dma_start(xs[:, :, :cs], xT_v[:, :, c0 : c0 + cs])
                # stage 1: hT = silu(w1T.T @ xT)  -> [I, cs], I on partitions
                hs = hpool.tile([P, IT, CHUNK], F32R, tag="hs")
                for it in range(IT):
                    ps1 = ps1pool.tile([P, CHUNK], F32, tag="ps1")
                    for hk in range(HK):
                        nc.tensor.matmul(
                            ps1[:, :cs],
                            w1s[:, hk, it * P : (it + 1) * P],
                            xs[:, hk, :cs],
                            start=(hk == 0),
                            stop=(hk == HK - 1),
                        )
                    # silu(z) = z * sigmoid(z); CoreSim has no Silu table,
                    # so build it from Sigmoid (ACT) + multiply (DVE)
                    sg = sgpool.tile([P, CHUNK], F32, tag="sg")
                    nc.scalar.activation(sg[:, :cs], ps1[:, :cs], AF.Sigmoid)
                    nc.vector.tensor_mul(
                        out=hs[:, it, :cs], in0=ps1[:, :cs], in1=sg[:, :cs]
                    )
                # stage 2: y = (hT.T @ w2T) * gate -> [cs, H], tokens on partitions
                for cc in range(cs // P):
                    h0 = 0
                    for hcs in h_chunks:
                        ps2 = ps2pool.tile([P, CHUNK], F32, tag="ps2")
                        for it in range(IT):
                            nc.tensor.matmul(
                                ps2[:, :hcs],
                                hs[:, it, cc * P : (cc + 1) * P],
                                w2s[:, it, h0 : h0 + hcs],
                                start=(it == 0),
                                stop=(it == IT - 1),
                            )
                        ys = ypool.tile([P, CHUNK], F32, tag="ys")
                        nc.vector.tensor_scalar_mul(
                            ys[:, :hcs], ps2[:, :hcs], gsb[:, ct0 + cc : ct0 + cc + 1]
                        )
                        nc.sync.dma_start(
                            y_v[ct0 + cc][:, h0 : h0 + hcs], ys[:, :hcs]
                        )
                        h0 += hcs
                c0 += cs
    nc.compile()
    global LAST_NC
    LAST_NC = nc
    return nc


def route(router_logits):
    """Host-side router: softmax -> top-2 -> renormalize.

    Returns (top2_idx [T,2] int64, top2_gate [T,2] float32)."""
    logits = np.asarray(router_logits, dtype=np.float32)
    m = logits.max(axis=-1, keepdims=True)
    ex = np.exp(logits - m)
    probs = ex / ex.sum(axis=-1, keepdims=True)
    order = np.argsort(-probs, axis=-1, kind="stable")[:, :TOPK]
    rows = np.arange(logits.shape[0])[:, None]
    topk_p = probs[rows, order]
    topk_p = topk_p / topk_p.sum(axis=-1, keepdims=True)
    return order, topk_p.astype(np.float32)


def kernel(x, router_logits, w1, w2):
    x = np.ascontiguousarray(np.asarray(x, dtype=np.float32))
    w1 = np.asarray(w1, dtype=np.float32)
    w2 = np.asarray(w2, dtype=np.float32)
    t = x.shape[0]

    top2_idx, top2_gate = route(router_logits)

    expert_tokens = []
    expert_gates = []
    for e in range(E):
        sel = np.nonzero(top2_idx == e)
        expert_tokens.append(sel[0])
        expert_gates.append(top2_gate[sel[0], sel[1]])
    counts = [len(ix) for ix in expert_tokens]
    C = max(P, -(-max(counts) // P) * P)
    n_ct = C // P

    nc = build_moe_expert_kernel(C)

    in_maps = []
    for e in range(E):
        cnt = counts[e]
        xT_e = np.zeros((H, C), dtype=np.float32)
        xT_e[:, :cnt] = x[expert_tokens[e]].T
        g = np.zeros(C, dtype=np.float32)
        g[:cnt] = expert_gates[e]
        in_maps.append(
            {
                "xT": xT_e,
                "w1T": np.ascontiguousarray(w1[e].T),
                "w2T": np.ascontiguousarray(w2[e].T),
                "gates": np.ascontiguousarray(g.reshape(n_ct, P).T),
            }
        )

    res = run_bass_kernel_spmd(nc, in_maps, core_ids=list(range(N_CORES)))

    out = np.zeros((t, H), dtype=np.float32)
    for e in range(E):
        cnt = counts[e]
        out[expert_tokens[e]] += res.results[e]["y"][:cnt]
    return out


# revision 16
# speedup vs baseline: 1.5970x; 1.5970x over previous
"""MoE (top-2 of 8 experts) Trainium2 kernel.

Strategy: expert-parallel across the 8 NeuronCores. The router
(softmax + top-2 over [T, 8] logits) is metadata computed on host to
build the dispatch; core e receives only the tokens routed to expert e
(gathered, transposed, zero-padded to a common capacity C) plus that
expert's weights, pre-transposed so the device does no transposes:

  core e inputs:  xT  [H, C]   = x[idx_e].T        (padded)
                  w1T [H, I]   = w1[e].T
                  w2T [I, H]   = w2[e].T
                  gates [128, C/128]  renormalized top-2 weight per token
  core e output:  y   [C, H]   = gate * (silu(x_e @ w1[e].T) @ w2[e].T)

On device (per core, all fp32 storage, float32r matmuls):
  stage 1: hT[i_tile, c_chunk] = silu(w1T.T @ xT)   (I on partitions)
  stage 2: y[c_tile, h_chunk]  = hT.T @ w2T, scaled per-partition by gate

The host then scatter-adds the two expert contributions per token.
"""

import numpy as np

import concourse.bass as bass
import concourse.mybir as mybir
from concourse import bacc
from concourse.tile import TileContext
from concourse.bass_utils import run_bass_kernel_spmd

T, H, I, E = 4096, 1024, 1408, 8
TOPK = 2
P = 128
CHUNK = 512
N_CORES = 8
F32 = mybir.dt.float32
F32R = mybir.dt.float32r
AF = mybir.ActivationFunctionType

# most recently built device program (for test harnesses / cost-model timing)
LAST_NC = None


def _chunk_sizes(C):
    """Split C into ceil(C/512) chunks, multiples of 128, as even as
    possible. Balanced chunks keep every stage-1 matmul's moving dim >=256
    (the fp32r full-rate threshold) instead of a slow ragged tail."""
    n = -(-C // CHUNK)
    base = (C // n) // P * P
    rem = (C - n * base) // P
    return [base + P if j < rem else base for j in range(n)]


def build_moe_expert_kernel(C, h=H, i_dim=I):
    """One-expert MLP over C gathered tokens. h, i_dim overridable for
    small-scale simulation tests; both must be multiples of 128, C a
    multiple of 128."""
    assert C % P == 0 and h % P == 0 and i_dim % P == 0
    HK = h // P
    IT = i_dim // P
    n_ct = C // P

    nc = bacc.Bacc("TRN2", target_bir_lowering=False, debug=False, num_devices=N_CORES)
    # Matmul inputs are stored as float32r (same 32-bit layout; the PE
    # rounds to its reduced internal precision). Typing the whole producer
    # chain as f32r satisfies the BIR verifier's rounding check.
    xT = nc.dram_tensor("xT", [h, C], F32R, kind="ExternalInput").ap()
    w1T = nc.dram_tensor("w1T", [h, i_dim], F32R, kind="ExternalInput").ap()
    w2T = nc.dram_tensor("w2T", [i_dim, h], F32R, kind="ExternalInput").ap()
    gates = nc.dram_tensor("gates", [P, n_ct], F32, kind="ExternalInput").ap()
    y = nc.dram_tensor("y", [C, h], F32, kind="ExternalOutput").ap()

    xT_v = xT.rearrange("(ho p) c -> p ho c", p=P)  # [128, HK, C]
    w1T_v = w1T.rearrange("(ho p) i -> p ho i", p=P)  # [128, HK, I]
    w2T_v = w2T.rearrange("(io p) h -> p io h", p=P)  # [128, IT, H]
    y_v = y.rearrange("(ct p) h -> ct p h", p=P)  # [n_ct, 128, H]

    h_chunks = _chunk_sizes(h)  # h-chunks for stage 2 output
    c_chunks = _chunk_sizes(C)
    max_cs = max(c_chunks)
    c_starts = [sum(c_chunks[:j]) for j in range(len(c_chunks))]
    with TileContext(nc) as tc:
        with (
            tc.tile_pool(name="wpool", bufs=1) as wpool,
            tc.tile_pool(name="xpool", bufs=3) as xpool,
            tc.tile_pool(name="hpool", bufs=2) as hpool,
            tc.tile_pool(name="ypool", bufs=2) as ypool,
            tc.tile_pool(name="sgpool", bufs=2) as sgpool,
            tc.tile_pool(name="ps1", bufs=4, space="PSUM") as ps1pool,
            tc.tile_pool(name="ps2", bufs=4, space="PSUM") as ps2pool,
        ):
            gsb = wpool.tile([P, n_ct], F32)
            w1s = wpool.tile([P, HK, i_dim], F32R)
            w2s = wpool.tile([P, IT, h], F32R)
            xs_tiles = {}

            def load_x(ci):
                xs = xpool.tile([P, HK, max_cs], F32R, tag="xs", name=f"xs{ci}")
                cs, c0 = c_chunks[ci], c_starts[ci]
                # per-hk DMAs so the first stage-1 matmul can start after
                # one h-slice instead of the whole chunk
                for hk in range(HK):
                    nc.sync.dma_start(xs[:, hk, :cs], xT_v[:, hk, c0 : c0 + cs])
                xs_tiles[ci] = xs

            def load_w1(it):
                nc.sync.dma_start(
                    w1s[:, :, it * P : (it + 1) * P],
                    w1T_v[:, :, it * P : (it + 1) * P],
                )

            # DMA issue order = consumption order. Interleave chunk-0 x
            # slices with the leading w1 i-tiles so the first stage-1
            # accumulation group starts after ~0.7 MB instead of ~6 MB;
            # then the rest of w1, the remaining x chunks, then w2 (per
            # h-half, consumed by stage 2).
            xs0 = xpool.tile([P, HK, max_cs], F32R, tag="xs", name="xs0")
            cs0 = c_chunks[0]
            for hk in range(HK):
                # the very first accumulation group reads w1[hk, it0] and
                # xs0[hk] in hk order — stream both at matching granularity
                nc.sync.dma_start(w1s[:, hk, 0:P], w1T_v[:, hk, 0:P])
                nc.sync.dma_start(xs0[:, hk, :cs0], xT_v[:, hk, 0:cs0])
            xs_tiles[0] = xs0
            for it in range(1, IT):
                load_w1(it)
            nc.sync.dma_start(gsb[:], gates[:])
            for ci in range(1, len(c_chunks)):
                load_x(ci)
            for h0, hcs in zip([sum(h_chunks[:j]) for j in range(len(h_chunks))], h_chunks):
                for it in range(IT):
                    nc.sync.dma_start(
                        w2s[:, it, h0 : h0 + hcs], w2T_v[:, it, h0 : h0 + hcs]
                    )

            hs_tiles = {}

            def stage1(ci):
                cs = c_chunks[ci]
                xs = xs_tiles[ci]
                # hT = silu(w1T.T @ xT)  -> [I, cs], I on partitions
                hs = hpool.tile([P, IT, max_cs], F32R, tag="hs", name=f"hs{ci}")
                for it in range(IT):
                    ps1 = ps1pool.tile([P, CHUNK], F32, tag="ps1")
                    for hk in range(HK):
                        nc.tensor.matmul(
                            ps1[:, :cs],
                            w1s[:, hk, it * P : (it + 1) * P],
                            xs[:, hk, :cs],
                            start=(hk == 0),
                            stop=(hk == HK - 1),
                        )
                    # silu(z) = z * sigmoid(z); CoreSim has no Silu table,
                    # so build it from Sigmoid (ACT) + multiply (DVE)
                    sg = sgpool.tile([P, CHUNK], F32, tag="sg")
                    nc.scalar.activation(sg[:, :cs], ps1[:, :cs], AF.Sigmoid)
                    nc.vector.tensor_mul(
                        out=hs[:, it, :cs], in0=ps1[:, :cs], in1=sg[:, :cs]
                    )
                hs_tiles[ci] = hs

            def stage2(ci):
                # y = (hT.T @ w2T) * gate -> [cs, H], tokens on partitions
                cs, ct0 = c_chunks[ci], c_starts[ci] // P
                hs = hs_tiles.pop(ci)
                for cc in range(cs // P):
                    h0 = 0
                    for hcs in h_chunks:
                        ps2 = ps2pool.tile([P, CHUNK], F32, tag="ps2")
                        for it in range(IT):
                            nc.tensor.matmul(
                                ps2[:, :hcs],
                                hs[:, it, cc * P : (cc + 1) * P],
                                w2s[:, it, h0 : h0 + hcs],
                                start=(it == 0),
                                stop=(it == IT - 1),
                            )
                        ys = ypool.tile([P, CHUNK], F32, tag="ys")
                        nc.vector.tensor_scalar_mul(
                            ys[:, :hcs], ps2[:, :hcs], gsb[:, ct0 + cc : ct0 + cc + 1]
                        )
                        nc.sync.dma_start(
                            y_v[ct0 + cc][:, h0 : h0 + hcs], ys[:, :hcs]
                        )
                        h0 += hcs

            # software pipeline: run stage 1 a chunk ahead so the PE has
            # stage-1 work for chunk i+1 while w2 is still streaming in
            stage1(0)
            for ci in range(1, len(c_chunks)):
                stage1(ci)
                stage2(ci - 1)
            stage2(len(c_chunks) - 1)
    nc.compile()
    global LAST_NC
    LAST_NC = nc
    return nc


def route(router_logits):
    """Host-side router: softmax -> top-2 -> renormalize.

    Returns (top2_idx [T,2] int64, top2_gate [T,2] float32)."""
    logits = np.asarray(router_logits, dtype=np.float32)
    m = logits.max(axis=-1, keepdims=True)
    ex = np.exp(logits - m)
    probs = ex / ex.sum(axis=-1, keepdims=True)
    order = np.argsort(-probs, axis=-1, kind="stable")[:, :TOPK]
    rows = np.arange(logits.shape[0])[:, None]
    topk_p = probs[rows, order]
    topk_p = topk_p / topk_p.sum(axis=-1, keepdims=True)
    return order, topk_p.astype(np.float32)


def kernel(x, router_logits, w1, w2):
    x = np.ascontiguousarray(np.asarray(x, dtype=np.float32))
    w1 = np.asarray(w1, dtype=np.float32)
    w2 = np.asarray(w2, dtype=np.float32)
    t = x.shape[0]

    top2_idx, top2_gate = route(router_logits)

    expert_tokens = []
    expert_gates = []
    for e in range(E):
        sel = np.nonzero(top2_idx == e)
        expert_tokens.append(sel[0])
        expert_gates.append(top2_gate[sel[0], sel[1]])
    counts = [len(ix) for ix in expert_tokens]
    C = max(P, -(-max(counts) // P) * P)
    n_ct = C // P

    nc = build_moe_expert_kernel(C)

    in_maps = []
    for e in range(E):
        cnt = counts[e]
        xT_e = np.zeros((H, C), dtype=np.float32)
        xT_e[:, :cnt] = x[expert_tokens[e]].T
        g = np.zeros(C, dtype=np.float32)
        g[:cnt] = expert_gates[e]
        in_maps.append(
            {
                "xT": xT_e,
                "w1T": np.ascontiguousarray(w1[e].T),
                "w2T": np.ascontiguousarray(w2[e].T),
                "gates": np.ascontiguousarray(g.reshape(n_ct, P).T),
            }
        )

    res = run_bass_kernel_spmd(nc, in_maps, core_ids=list(range(N_CORES)))

    out = np.zeros((t, H), dtype=np.float32)
    for e in range(E):
        cnt = counts[e]
        out[expert_tokens[e]] += res.results[e]["y"][:cnt]
    return out


# revision 30
# speedup vs baseline: 1.5986x; 1.0010x over previous
"""MoE (top-2 of 8 experts) Trainium2 kernel.

Strategy: expert-parallel across the 8 NeuronCores. The router
(softmax + top-2 over [T, 8] logits) is metadata computed on host to
build the dispatch; core e receives only the tokens routed to expert e
(gathered, transposed, zero-padded to a common capacity C) plus that
expert's weights, pre-transposed so the device does no transposes:

  core e inputs:  xT  [H, C]   = x[idx_e].T        (padded)
                  w1T [H, I]   = w1[e].T
                  w2T [I, H]   = w2[e].T
                  gates [128, C/128]  renormalized top-2 weight per token
  core e output:  y   [C, H]   = gate * (silu(x_e @ w1[e].T) @ w2[e].T)

On device (per core, all fp32 storage, float32r matmuls):
  stage 1: hT[i_tile, c_chunk] = silu(w1T.T @ xT)   (I on partitions)
  stage 2: y[c_tile, h_chunk]  = hT.T @ w2T, scaled per-partition by gate

The host then scatter-adds the two expert contributions per token.
"""

import numpy as np

import concourse.bass as bass
import concourse.mybir as mybir
from concourse import bacc
from concourse.tile import TileContext
from concourse.bass_utils import run_bass_kernel_spmd

T, H, I, E = 4096, 1024, 1408, 8
TOPK = 2
P = 128
CHUNK = 512
N_CORES = 8
F32 = mybir.dt.float32
F32R = mybir.dt.float32r
AF = mybir.ActivationFunctionType

# most recently built device program (for test harnesses / cost-model timing)
LAST_NC = None


def _chunk_sizes(C):
    """Split C into ceil(C/512) chunks, multiples of 128, as even as
    possible. Balanced chunks keep every stage-1 matmul's moving dim >=256
    (the fp32r full-rate threshold) instead of a slow ragged tail."""
    n = -(-C // CHUNK)
    base = (C // n) // P * P
    rem = (C - n * base) // P
    return [base + P if j < rem else base for j in range(n)]


def build_moe_expert_kernel(C, h=H, i_dim=I):
    """One-expert MLP over C gathered tokens. h, i_dim overridable for
    small-scale simulation tests; both must be multiples of 128, C a
    multiple of 128."""
    assert C % P == 0 and h % P == 0 and i_dim % P == 0
    HK = h // P
    IT = i_dim // P
    n_ct = C // P

    nc = bacc.Bacc("TRN2", target_bir_lowering=False, debug=False, num_devices=N_CORES)
    # Matmul inputs are stored as float32r (same 32-bit layout; the PE
    # rounds to its reduced internal precision). Typing the whole producer
    # chain as f32r satisfies the BIR verifier's rounding check.
    xT = nc.dram_tensor("xT", [h, C], F32R, kind="ExternalInput").ap()
    w1T = nc.dram_tensor("w1T", [h, i_dim], F32R, kind="ExternalInput").ap()
    w2T = nc.dram_tensor("w2T", [i_dim, h], F32R, kind="ExternalInput").ap()
    gates = nc.dram_tensor("gates", [P, n_ct], F32, kind="ExternalInput").ap()
    y = nc.dram_tensor("y", [C, h], F32, kind="ExternalOutput").ap()

    xT_v = xT.rearrange("(ho p) c -> p ho c", p=P)  # [128, HK, C]
    w1T_v = w1T.rearrange("(ho p) i -> p ho i", p=P)  # [128, HK, I]
    w2T_v = w2T.rearrange("(io p) h -> p io h", p=P)  # [128, IT, H]
    y_v = y.rearrange("(ct p) h -> ct p h", p=P)  # [n_ct, 128, H]

    h_chunks = _chunk_sizes(h)  # h-chunks for stage 2 output
    c_chunks = _chunk_sizes(C)
    max_cs = max(c_chunks)
    c_starts = [sum(c_chunks[:j]) for j in range(len(c_chunks))]
    # per-partition SBUF bytes: weights + 3 x-chunk bufs + 2 h bufs + sg;
    # give the y pool 4 bufs when it still fits the 192 KB budget
    fixed = 4 * (HK * i_dim + IT * h + 3 * HK * max_cs + 2 * IT * max_cs + 2 * CHUNK)
    y_bufs = 4 if fixed + 4 * 4 * CHUNK < 190 * 1024 else 2
    with TileContext(nc) as tc:
        with (
            tc.tile_pool(name="wpool", bufs=1) as wpool,
            tc.tile_pool(name="xpool", bufs=3) as xpool,
            tc.tile_pool(name="hpool", bufs=2) as hpool,
            tc.tile_pool(name="ypool", bufs=y_bufs) as ypool,
            tc.tile_pool(name="sgpool", bufs=2) as sgpool,
            tc.tile_pool(name="ps1", bufs=4, space="PSUM") as ps1pool,
            tc.tile_pool(name="ps2", bufs=4, space="PSUM") as ps2pool,
        ):
            gsb = wpool.tile([P, n_ct], F32)
            w1s = wpool.tile([P, HK, i_dim], F32R)
            w2s = wpool.tile([P, IT, h], F32R)
            xs_tiles = {}

            def load_x(ci, split=True):
                # per-hk DMAs deliver the chunk incrementally so stage-1
                # groups can start before the whole chunk lands
                xs = xpool.tile([P, HK, max_cs], F32R, tag="xs", name=f"xs{ci}")
                cs, c0 = c_chunks[ci], c_starts[ci]
                if split:
                    for hk in range(HK):
                        nc.sync.dma_start(xs[:, hk, :cs], xT_v[:, hk, c0 : c0 + cs])
                else:
                    nc.sync.dma_start(xs[:, :, :cs], xT_v[:, :, c0 : c0 + cs])
                xs_tiles[ci] = xs

            def load_w1(it):
                nc.sync.dma_start(
                    w1s[:, :, it * P : (it + 1) * P],
                    w1T_v[:, :, it * P : (it + 1) * P],
                )

            # DMA issue order = consumption order. Interleave chunk-0 x
            # slices with the leading w1 i-tiles so the first stage-1
            # accumulation group starts after ~0.7 MB instead of ~6 MB;
            # then the rest of w1, the remaining x chunks, then w2 (per
            # h-half, consumed by stage 2).
            xs0 = xpool.tile([P, HK, max_cs], F32R, tag="xs", name="xs0")
            cs0 = c_chunks[0]
            load_w1(0)
            for hk in range(HK):
                nc.sync.dma_start(xs0[:, hk, :cs0], xT_v[:, hk, 0:cs0])
                if hk == 1:
                    load_w1(1)
            xs_tiles[0] = xs0
            nc.sync.dma_start(gsb[:], gates[:])
            for it in range(2, IT):
                load_w1(it)
            # w2 per h-half per i-tile: stage 2 consumes one h-chunk across
            # i-tiles in order, so fine-grained delivery unblocks each
            # accumulation group as early as possible
            h_starts = [sum(h_chunks[:j]) for j in range(len(h_chunks))]
            for ci in range(1, len(c_chunks)):
                load_x(ci)
            for h0, hcs in zip(h_starts, h_chunks):
                for it in range(IT):
                    nc.sync.dma_start(
                        w2s[:, it, h0 : h0 + hcs], w2T_v[:, it, h0 : h0 + hcs]
                    )

            hs_tiles = {}

            def stage1(ci):
                cs = c_chunks[ci]
                xs = xs_tiles[ci]
                # hT = silu(w1T.T @ xT)  -> [I, cs], I on partitions
                hs = hpool.tile([P, IT, max_cs], F32R, tag="hs", name=f"hs{ci}")
                for it in range(IT):
                    ps1 = ps1pool.tile([P, CHUNK], F32, tag="ps1")
                    for hk in range(HK):
                        nc.tensor.matmul(
                            ps1[:, :cs],
                            w1s[:, hk, it * P : (it + 1) * P],
                            xs[:, hk, :cs],
                            start=(hk == 0),
                            stop=(hk == HK - 1),
                        )
                    # silu(z) = z * sigmoid(z); CoreSim has no Silu table,
                    # so build it from Sigmoid (ACT) + multiply (DVE)
                    sg = sgpool.tile([P, CHUNK], F32, tag="sg")
                    nc.scalar.activation(sg[:, :cs], ps1[:, :cs], AF.Sigmoid)
                    nc.vector.tensor_mul(
                        out=hs[:, it, :cs], in0=ps1[:, :cs], in1=sg[:, :cs]
                    )
                hs_tiles[ci] = hs

            def stage2(ci):
                # y = (hT.T @ w2T) * gate -> [cs, H], tokens on partitions
                cs, ct0 = c_chunks[ci], c_starts[ci] // P
                hs = hs_tiles.pop(ci)
                for cc in range(cs // P):
                    h0 = 0
                    for hcs in h_chunks:
                        ps2 = ps2pool.tile([P, CHUNK], F32, tag="ps2")
                        for it in range(IT):
                            nc.tensor.matmul(
                                ps2[:, :hcs],
                                hs[:, it, cc * P : (cc + 1) * P],
                                w2s[:, it, h0 : h0 + hcs],
                                start=(it == 0),
                                stop=(it == IT - 1),
                            )
                        ys = ypool.tile([P, CHUNK], F32, tag="ys")
                        nc.vector.tensor_scalar_mul(
                            ys[:, :hcs],
                            ps2[:, :hcs],
                            gsb[:, ct0 + cc : ct0 + cc + 1],
                        )
                        nc.sync.dma_start(y_v[ct0 + cc][:, h0 : h0 + hcs], ys[:, :hcs])
                        h0 += hcs

            # software pipeline: run stage 1 a chunk ahead so the PE has
            # stage-1 work for chunk i+1 while w2 is still streaming in
            stage1(0)
            for ci in range(1, len(c_chunks)):
                stage1(ci)
                stage2(ci - 1)
            stage2(len(c_chunks) - 1)
    nc.compile()
    global LAST_NC
    LAST_NC = nc
    return nc


def route(router_logits):
    """Host-side router: softmax -> top-2 -> renormalize.

    Returns (top2_idx [T,2] int64, top2_gate [T,2] float32)."""
    logits = np.asarray(router_logits, dtype=np.float32)
    m = logits.max(axis=-1, keepdims=True)
    ex = np.exp(logits - m)
    probs = ex / ex.sum(axis=-1, keepdims=True)
    order = np.argsort(-probs, axis=-1, kind="stable")[:, :TOPK]
    rows = np.arange(logits.shape[0])[:, None]
    topk_p = probs[rows, order]
    topk_p = topk_p / topk_p.sum(axis=-1, keepdims=True)
    return order, topk_p.astype(np.float32)


def kernel(x, router_logits, w1, w2):
    x = np.ascontiguousarray(np.asarray(x, dtype=np.float32))
    w1 = np.asarray(w1, dtype=np.float32)
    w2 = np.asarray(w2, dtype=np.float32)
    t = x.shape[0]

    top2_idx, top2_gate = route(router_logits)

    expert_tokens = []
    expert_gates = []
    for e in range(E):
        sel = np.nonzero(top2_idx == e)
        expert_tokens.append(sel[0])
        expert_gates.append(top2_gate[sel[0], sel[1]])
    counts = [len(ix) for ix in expert_tokens]
    C = max(P, -(-max(counts) // P) * P)
    n_ct = C // P

    nc = build_moe_expert_kernel(C)

    in_maps = []
    for e in range(E):
        cnt = counts[e]
        xT_e = np.zeros((H, C), dtype=np.float32)
        xT_e[:, :cnt] = x[expert_tokens[e]].T
        g = np.zeros(C, dtype=np.float32)
        g[:cnt] = expert_gates[e]
        in_maps.append(
            {
                "xT": xT_e,
                "w1T": np.ascontiguousarray(w1[e].T),
                "w2T": np.ascontiguousarray(w2[e].T),
                "gates": np.ascontiguousarray(g.reshape(n_ct, P).T),
            }
        )

    res = run_bass_kernel_spmd(nc, in_maps, core_ids=list(range(N_CORES)))
    if not all(np.isfinite(r["y"]).all() for r in res.results):
        # one retry in case of a transient device fault
        res = run_bass_kernel_spmd(nc, in_maps, core_ids=list(range(N_CORES)))

    out = np.zeros((t, H), dtype=np.float32)
    for e in range(E):
        cnt = counts[e]
        out[expert_tokens[e]] += res.results[e]["y"][:cnt]
    return out
